# revision 22
# baseline (speedup 1.0000x reference)
"""LoRA-injected 3x3 conv (MoE-routed adapters), Trainium2 Bass kernel.

Strategy (Winograd F(2x2, 3x3), bf16):
 - Host: merge each sample's LoRA adapter into the base conv weight
   (exact low-rank merge), Winograd-transform the merged weights
   (G W G^T -> 16 positions) and the input (B^T d B over 4x4 tiles,
   stride 2 -> 32x32 tiles/sample), both cast to bf16.
 - Device (per core, 2 samples): pure GEMM - for each of 16 Winograd
   positions, ytil[pos] = Wtil[pos]^T @ xtil[pos] with K=Cin=320
   (chunks 128/128/64) accumulated in PSUM, N=512 tiles (half image).
   ScalarE evacuates PSUM->SBUF as bf16; VectorE applies the output
   transform A^T ytil A (all +-1 coeffs, contiguous aligned bf16 ops
   so the DVE runs in 2x packed mode) with the bias folded in; result
   DMA'd out as bf16.
 - This halves PE work vs direct conv (295K vs 565K cycles/core).
"""

import sys

for _p in ("/opt/trn_rl_repo",):
    if _p not in sys.path:
        sys.path.insert(0, _p)

import numpy as np
import ml_dtypes

BF16 = ml_dtypes.bfloat16

B, CIN, COUT, H, W = 16, 320, 320, 64, 64
R, NUM_LORAS, LORA_STRIDE, SCALE = 4, 50, 4, 1.0
NCORES = 8
BLOC = B // NCORES          # samples per core
NPOS = 16                   # winograd positions (4x4)
TPH = 512                   # tiles per half-image (16 tile-rows x 32)
KCH = [(0, 128), (128, 128), (256, 64)]   # Cin chunks
CCH = [(0, 128), (128, 128), (256, 64)]   # Cout chunks
HWFLAT = H * W

_NC_CACHE = {}


def _build_nc():
    import concourse.bacc as bacc
    import concourse.bass as bass
    import concourse.mybir as mybir
    from concourse import tile

    f32 = mybir.dt.float32
    bf16 = mybir.dt.bfloat16
    AOp = mybir.AluOpType

    nc = bacc.Bacc(None, target_bir_lowering=False)

    # winograd-domain input, ci chunks 0/1: [sample*2+half, ci, a, b*512]
    xt_d = nc.dram_tensor("xt", [BLOC * 2, 256, 4, 4 * TPH], bf16, kind="ExternalInput")
    # ci tail chunk (64), pos-paired: partitions 0:64 = even b, 64:128 = odd b
    xt2_d = nc.dram_tensor("xt2", [BLOC * 2, 128, 4, 2 * TPH], bf16, kind="ExternalInput")
    # winograd-domain merged weights: [sample, ci, pos*co]
    wt_d = nc.dram_tensor("wt", [BLOC, 256, NPOS * COUT], bf16, kind="ExternalInput")
    wt2_d = nc.dram_tensor("wt2", [BLOC, 128, 8 * COUT], bf16, kind="ExternalInput")
    y_d = nc.dram_tensor("y", [BLOC, COUT, HWFLAT], bf16, kind="ExternalOutput")

    with tile.TileContext(nc) as tc:
        with (
            tc.tile_pool(name="wp", bufs=2) as wpool,
            tc.tile_pool(name="xp", bufs=2) as xpool,
            tc.tile_pool(name="ytp", bufs=2) as ytpool,
            tc.tile_pool(name="up", bufs=2) as upool,
            tc.tile_pool(name="osp", bufs=2) as ospool,
            tc.tile_pool(name="cst", bufs=1) as cpool,
            tc.tile_pool(name="acc", bufs=2, space=bass.MemorySpace.PSUM) as pspool,
        ):
            # HAM warm-up: ~5us of dense dummy matmuls at t=0 (overlapping the
            # initial DMAs) so the PE clock gate releases to 2.4 GHz before
            # the real matmul stream starts.
            wu = cpool.tile([128, TPH], bf16, tag="wu")
            nc.vector.memset(wu[:], 0.0)
            wups = pspool.tile([128, 4, TPH], f32, tag="ps")
            for i in range(56):
                nc.tensor.matmul(
                    wups[:, i % 4, :], wu[:, :128], wu[:], start=True, stop=True
                )

            qrr = [0]
            QUEUES = (nc.sync, nc.scalar)

            def qdma(out, in_):
                QUEUES[qrr[0] % 2].dma_start(out=out, in_=in_)
                qrr[0] += 1

            for s in range(BLOC):
                # weights + h0 inputs issued in strict consumption order,
                # round-robin across the 3 DMA queues
                wts = [[None] * 3 for _ in range(2)]
                w2s = [None] * 3
                xts_h = [None, None]
                x2s_h = [None, None]

                def load_w(cc):
                    o0, osz = CCH[cc]
                    for kc, (c0, csz) in enumerate(KCH[:2]):
                        wtile = wpool.tile([csz, NPOS, osz], bf16, tag=f"w{kc}c{cc}")
                        qdma(
                            wtile[:],
                            wt_d[s, c0 : c0 + csz, :]
                            .rearrange("p (t c) -> p t c", c=COUT)[:, :, o0 : o0 + osz],
                        )
                        wts[kc][cc] = wtile
                    w2tile = wpool.tile([128, 8, osz], bf16, tag=f"w2c{cc}")
                    qdma(
                        w2tile[:],
                        wt2_d[s, :, :]
                        .rearrange("p (t c) -> p t c", c=COUT)[:, :, o0 : o0 + osz],
                    )
                    w2s[cc] = w2tile

                def load_x(h, a):
                    sh = s * 2 + h
                    for kc, (c0, csz) in enumerate(KCH[:2]):
                        xtile = xpool.tile([csz, 4, TPH], bf16, tag=f"x{kc}a{a}")
                        qdma(
                            xtile[:],
                            xt_d[sh, c0 : c0 + csz, a, :]
                            .rearrange("p (b n) -> p b n", n=TPH),
                        )
                        xts_h[h][kc][a] = xtile
                    x2tile = xpool.tile([128, 2, TPH], bf16, tag=f"x2a{a}")
                    qdma(
                        x2tile[:],
                        xt2_d[sh, :, a, :].rearrange("p (q n) -> p q n", n=TPH),
                    )
                    x2s_h[h][a] = x2tile

                xts_h = [[[None] * 4 for _ in range(2)] for _ in range(2)]
                x2s_h = [[None] * 4 for _ in range(2)]
                load_w(0)
                load_x(0, 0)
                load_w(1)
                load_x(0, 1)
                load_w(2)
                load_x(0, 2)
                load_x(0, 3)
                for a in range(4):
                    load_x(1, a)

                for h in range(2):
                    xts = [[xts_h[h][kc][a] for a in range(4)] for kc in range(2)]
                    x2s = x2s_h[h]

                    for cc, (o0, osz) in enumerate(CCH):
                        # yt: bf16 copies of the 16 position GEMM results
                        yt = ytpool.tile([128, 4, 4, TPH], bf16, tag="yt")
                        for a in range(4):
                            ps = pspool.tile([128, 4, TPH], f32, tag="ps")
                            for b in range(4):
                                for kc in range(2):
                                    nc.tensor.matmul(
                                        ps[:osz, b, :],
                                        wts[kc][cc][:, a * 4 + b, :],
                                        xts[kc][a][:, b, :],
                                        start=(kc == 0),
                                        stop=False,
                                    )
                            # ci tail (K=64): even/odd positions packed into
                            # row-groups 0-1 / 2-3 -> pairs run concurrently
                            for p in range(2):
                                nc.tensor.matmul(
                                    ps[:osz, 2 * p, :],
                                    w2s[cc][0:64, a * 2 + p, :],
                                    x2s[a][0:64, p, :],
                                    start=False,
                                    stop=True,
                                )
                                nc.tensor.matmul(
                                    ps[:osz, 2 * p + 1, :],
                                    w2s[cc][64:128, a * 2 + p, :],
                                    x2s[a][64:128, p, :],
                                    start=False,
                                    stop=True,
                                )
                            # evacuate 4 banks in one activation
                            nc.scalar.activation(
                                yt[:osz, a, :, :],
                                ps[:osz],
                                mybir.ActivationFunctionType.Identity,
                            )

                        # output transform, all on VectorE in packed bf16.
                        # col pass (over b): u0 = y0+y1+y2 ; u1 = y1-y2-y3
                        u = upool.tile([128, 4, 16, 2, 32], bf16, tag="u")
                        tmp = upool.tile([128, 4, 16, 32], bf16, tag="tmp")
                        nc.vector.tensor_add(
                            tmp[:osz], yt[:osz, :, 0, :].rearrange("p a (i j) -> p a i j", j=32),
                            yt[:osz, :, 1, :].rearrange("p a (i j) -> p a i j", j=32),
                        )
                        nc.vector.tensor_add(
                            u[:osz, :, :, 0, :], tmp[:osz],
                            yt[:osz, :, 2, :].rearrange("p a (i j) -> p a i j", j=32),
                        )
                        tmp2 = upool.tile([128, 4, 16, 32], bf16, tag="tmp")
                        nc.vector.tensor_sub(
                            tmp2[:osz], yt[:osz, :, 1, :].rearrange("p a (i j) -> p a i j", j=32),
                            yt[:osz, :, 2, :].rearrange("p a (i j) -> p a i j", j=32),
                        )
                        nc.vector.tensor_sub(
                            u[:osz, :, :, 1, :], tmp2[:osz],
                            yt[:osz, :, 3, :].rearrange("p a (i j) -> p a i j", j=32),
                        )
                        # row pass (over a): y_even = u[0]+u[1]+u[2],
                        # y_odd = u[1]-u[2]-u[3]; bias is added on the host.
                        # Output stays PLANAR [dh, i, dw, j] (contiguous DVE
                        # writes); the host un-interleaves rows/cols at decode.
                        ys = ospool.tile([128, 2, 16, 64], bf16, tag="ys")
                        te = upool.tile([128, 16, 64], bf16, tag="te")
                        nc.vector.tensor_add(
                            te[:osz],
                            u[:osz, 0].rearrange("p i d j -> p i (d j)"),
                            u[:osz, 1].rearrange("p i d j -> p i (d j)"),
                        )
                        nc.vector.tensor_add(
                            ys[:osz, 0],
                            te[:osz],
                            u[:osz, 2].rearrange("p i d j -> p i (d j)"),
                        )
                        to = upool.tile([128, 16, 64], bf16, tag="te")
                        nc.vector.tensor_sub(
                            to[:osz],
                            u[:osz, 1].rearrange("p i d j -> p i (d j)"),
                            u[:osz, 2].rearrange("p i d j -> p i (d j)"),
                        )
                        nc.vector.tensor_sub(
                            ys[:osz, 1],
                            to[:osz],
                            u[:osz, 3].rearrange("p i d j -> p i (d j)"),
                        )
                        nc.gpsimd.dma_start(
                            out=y_d[s, o0 : o0 + osz, h * 2048 : (h + 1) * 2048],
                            in_=ys[:osz].rearrange("p d i w -> p (d i w)"),
                        )

    nc.compile()
    return nc


def _get_nc():
    if "nc" not in _NC_CACHE:
        _NC_CACHE["nc"] = _build_nc()
    return _NC_CACHE["nc"]


def _prep_inputs(x, conv_w, conv_b, down_w, up_w, lora_id):
    x = np.asarray(x, dtype=np.float32)
    conv_w = np.asarray(conv_w, dtype=np.float32)
    conv_b = np.asarray(conv_b, dtype=np.float32)
    down_w = np.asarray(down_w, dtype=np.float32)
    up_w = np.asarray(up_w, dtype=np.float32)
    idx = np.asarray(lora_id).astype(np.int64) // LORA_STRIDE
    active = (idx >= 0).astype(np.float32)
    safe = np.clip(idx, 0, NUM_LORAS - 1)

    # Exact LoRA merge: W_eff = conv_w + scale*active * (up @ down)
    lora = np.matmul(up_w[safe], down_w[safe].reshape(B, R, -1))
    lora = lora.reshape(B, COUT, CIN, 3, 3)
    weff = conv_w[None] + (SCALE * active)[:, None, None, None, None] * lora

    # Winograd weight transform: wt[b, ci, (a,bb), co] = G Weff G^T
    G = np.array(
        [[1, 0, 0], [0.5, 0.5, 0.5], [0.5, -0.5, 0.5], [0, 0, 1]], dtype=np.float32
    )
    wtf = np.einsum("ak,bl,noikl->niabo", G, G, weff, optimize=True)  # [B,ci,4,4,co]
    wt = np.ascontiguousarray(wtf[:, :256].reshape(B, 256, NPOS * COUT)).astype(BF16)
    # ci tail, pos-paired into partition halves: [B, 128, (a,p), co]
    wt2f = np.empty((B, 128, 4, 2, COUT), np.float32)
    wt2f[:, 0:64] = wtf[:, 256:, :, 0::2]
    wt2f[:, 64:128] = wtf[:, 256:, :, 1::2]
    wt2 = np.ascontiguousarray(wt2f.reshape(B, 128, 8 * COUT)).astype(BF16)

    # Winograd input transform: B^T d B over 4x4 tiles, stride 2
    xp = np.zeros((B, CIN, 66, 66), np.float32)
    xp[:, :, 1:65, 1:65] = x
    d0 = xp[:, :, 0:64:2]
    d1 = xp[:, :, 1:65:2]
    d2 = xp[:, :, 2:66:2]
    d3 = xp[:, :, 3:66:2]
    ms = [d0 - d2, d1 + d2, d2 - d1, d1 - d3]          # [B, CIN, 32, 66] each
    xt = np.empty((B, CIN, 4, 4, 32, 32), np.float32)
    for a, ma in enumerate(ms):
        e0 = ma[..., 0:64:2]
        e1 = ma[..., 1:65:2]
        e2 = ma[..., 2:66:2]
        e3 = ma[..., 3:66:2]
        xt[:, :, a, 0] = e0 - e2
        xt[:, :, a, 1] = e1 + e2
        xt[:, :, a, 2] = e2 - e1
        xt[:, :, a, 3] = e1 - e3
    # -> [B, half, CIN, a, b, 16, 32]
    xt = xt.reshape(B, CIN, 4, 4, 2, 16, 32).transpose(0, 4, 1, 2, 3, 5, 6)
    xt_m = np.ascontiguousarray(xt[:, :, :256]).reshape(B * 2, 256, 4, 4 * TPH).astype(BF16)
    # ci tail, pos-paired: [B, half, 128, a, p, tph]
    tail = xt[:, :, 256:]                      # [B, 2, 64, 4, 4, 16, 32]
    xt2f = np.empty((B, 2, 128, 4, 2, TPH), np.float32)
    xt2f[:, :, 0:64] = tail[:, :, :, :, 0::2].reshape(B, 2, 64, 4, 2, TPH)
    xt2f[:, :, 64:128] = tail[:, :, :, :, 1::2].reshape(B, 2, 64, 4, 2, TPH)
    xt2 = np.ascontiguousarray(xt2f).reshape(B * 2, 128, 4, 2 * TPH).astype(BF16)

    in_maps = [
        {
            "xt": np.ascontiguousarray(xt_m[c * BLOC * 2 : (c + 1) * BLOC * 2]),
            "xt2": np.ascontiguousarray(xt2[c * BLOC * 2 : (c + 1) * BLOC * 2]),
            "wt": np.ascontiguousarray(wt[c * BLOC : (c + 1) * BLOC]),
            "wt2": np.ascontiguousarray(wt2[c * BLOC : (c + 1) * BLOC]),
        }
        for c in range(NCORES)
    ]
    return in_maps


def run_device(in_maps, trace=False, tmpdir=None):
    from concourse.bass_utils import run_bass_kernel_spmd

    nc = _get_nc()
    return run_bass_kernel_spmd(
        nc, in_maps, list(range(NCORES)), trace=trace, tmpdir=tmpdir
    )


def decode_y(out, conv_b):
    """Device y layout per sample: flat[h, dh, i, dw, j] with
    row = 32h + 2i + dh, col = 2j + dw. Bias is added here (host side)."""
    y = np.concatenate([out.results[c]["y"] for c in range(NCORES)], axis=0)
    y = y.astype(np.float32).reshape(B, COUT, 2, 2, 16, 2, 32)
    y = y.transpose(0, 1, 2, 4, 3, 6, 5)  # -> [B, co, h, i, dh, j, dw]
    y = np.ascontiguousarray(y.reshape(B, COUT, H, W))
    y += np.asarray(conv_b, np.float32)[None, :, None, None]
    return y


def kernel(x, conv_w, conv_b, down_w, up_w, lora_id):
    in_maps = _prep_inputs(x, conv_w, conv_b, down_w, up_w, lora_id)
    out = run_device(in_maps)
    return decode_y(out, conv_b)
